# revision 29
# baseline (speedup 1.0000x reference)
"""Trainium2 Bass kernel for the CGF tree-GRU problem.

Problem: 3-level complete 8-ary tree GRU (torch GRU cell convention).
  Level 3: 64 nodes x 8 embedded leaf children, h0 = 0
  Level 2:  8 nodes x 8 children (level-3 outputs), h0 = mean of children h
  Level 1:  1 node  x 8 children (level-2 outputs), h0 = mean of children h
  Output: mean over the 8 step outputs of the root GRU. D = 512.

Distribution choice: the computation is ONE serial chain of 24 GRU steps
(8 per level; levels strictly dependent).  Each step is dominated by moving
W_hh (1536x512) through the PE array, independent of the node-batch size, so
sharding the node batch across cores saves nothing, and sharding the hidden
dim requires a per-step all-gather whose latency exceeds a whole step.  The
kernel is therefore replicated on all 8 cores (SPMD, identical inputs); core
0's output is returned.

Layout: everything lives TRANSPOSED on chip - gate/hidden dims on the 128
partitions (4 or 12 tiles of 128), batch on the free dim.  This makes GRU
biases per-partition scalars, halves DVE cost vs. the natural layout, and
removes all transposes: the recurrent matmul gh^T = W_hh @ h^T consumes h^T
directly, and each level's mean-output feeds the next level's input matmul
without reshaping.

Precision: matmul operands are bf16 (PSUM accumulation is fp32); everything
else - the carried state h, gi, gates, biases - stays fp32.  A bf16 shadow
of h feeds the matmuls: storing the state itself in bf16 costs ~8e-3
scale-relative error while bf16 matmul inputs only cost ~1e-3 (measured).

Scheduling shape per step: the 48 W_hh matmuls are a pure LDW-rate-bound
burst (r,z gate slices first) into four single-bank PSUM tiles, so the
chunked gi+gh adds / sigmoids stream on DVE/ACT behind the burst without
tripping same-bank PE-write/DVE-read serialization.  The fp32 state update
and the output accumulation run during the NEXT step's burst on the
otherwise-idle gpsimd engine - only the bf16 shadow write gates the next
matmul burst.
"""

import numpy as np

import concourse.bacc as bacc
import concourse.mybir as mybir
from concourse.tile import TileContext
from concourse.bass_utils import run_bass_kernel_spmd

AF = mybir.ActivationFunctionType
OP = mybir.AluOpType
FP = mybir.dt.float32
BF = mybir.dt.bfloat16

P = 128          # partitions
D = 512          # hidden size
KT = D // P      # 4 k-tiles (contraction)
G = 3 * D        # 1536 gate dims
MT = G // P      # 12 m-tiles (gate rows)
A = 8            # tree arity == sequence length per level
NB = 64          # level-3 node count
T = 8            # steps per level
N_CORES = 8

# bf16 blob layout: [xt(2048) | wit(6144) | wht(6144)]
O_XT = 0
O_WIT = O_XT + KT * T * NB
O_WHT = O_WIT + MT * KT * P
B16_COLS = O_WHT + MT * KT * P
# fp32 blob: [gb(12) | bhn(4) | bhnb(256)]
B32_COLS = MT + KT + KT * NB

_BUILT = None  # cached Bass module


def _v(ap, g):
    """View a 2-D [P, g*b] AP as [P, g, b]."""
    return ap.rearrange("p (g b) -> p g b", g=g)


def _build_nc():
    nc = bacc.Bacc()

    blob16 = nc.declare_dram_parameter("blob16", [P, B16_COLS], BF, isOutput=False)
    blob32 = nc.declare_dram_parameter("blob32", [P, B32_COLS], FP, isOutput=False)
    outp = nc.declare_dram_parameter("out", [P, KT], FP, isOutput=True)

    with TileContext(nc) as tc:
        with (
            tc.tile_pool(name="const", bufs=1) as cpool,
            tc.tile_pool(name="state", bufs=1) as spool,
            tc.tile_pool(name="work", bufs=2) as wpool,
            tc.tile_pool(name="pg", bufs=4, space="PSUM") as gpool,
            tc.tile_pool(name="prza", bufs=1, space="PSUM") as rzapool,
            tc.tile_pool(name="przb", bufs=1, space="PSUM") as rzbpool,
            tc.tile_pool(name="pna", bufs=1, space="PSUM") as napool,
            tc.tile_pool(name="pnb", bufs=1, space="PSUM") as nbpool,
        ):
            # Warm the activation tables before anything else: the lazy
            # ACT_TABLE_LOADs otherwise land mid-kernel and stall the first
            # sigmoid/tanh by >1us each.
            warm = cpool.tile([P, 8], FP)
            nc.vector.memset(warm[:, :], 0.0)
            for fn in (AF.Identity, AF.Sigmoid, AF.Tanh):
                nc.scalar.activation(warm[:, :], warm[:, :], fn)

            # Chunked input DMA (a wide DMA fans out over many HW-DGE queues
            # and blows the per-instruction sync-wait budget downstream; all
            # consumer slices stay within one 1024-col chunk).  Chunks
            # alternate between the two HWDGE-capable engines' rings so two
            # transfers are in flight at once.
            b32_sb = cpool.tile([P, B32_COLS], FP)
            nc.scalar.dma_start(out=b32_sb[:], in_=blob32[:, :])
            b16_sb = cpool.tile([P, B16_COLS], BF)
            for i, c0 in enumerate(range(0, B16_COLS, 1024)):
                c1 = min(c0 + 1024, B16_COLS)
                eng = nc.sync if i % 2 == 0 else nc.scalar
                eng.dma_start(out=b16_sb[:, c0:c1], in_=blob16[:, c0:c1])

            xt_sb = b16_sb[:, O_XT : O_XT + KT * T * NB]
            wit_sb = b16_sb[:, O_WIT : O_WIT + MT * KT * P]
            wht_sb = b16_sb[:, O_WHT : O_WHT + MT * KT * P]
            gb_sb = b32_sb[:, 0:MT]
            bhn_sb = b32_sb[:, MT : MT + KT]
            bhnb_sb = b32_sb[:, MT + KT : MT + KT + KT * NB]

            def compute_gi(gi_tile, rhs_of_k, ncols):
                """gi^T = W_ih @ x^T + combined bias (fp32 out), m-major."""
                for m in range(MT):
                    ps = gpool.tile([P, ncols], FP, tag="gi_ps")
                    for k in range(KT):
                        nc.tensor.matmul(
                            ps[:, :],
                            lhsT=wit_sb[:, (m * KT + k) * P : (m * KT + k + 1) * P],
                            rhs=rhs_of_k(k),
                            start=(k == 0),
                            stop=(k == KT - 1),
                        )
                    nc.scalar.activation(
                        gi_tile[:, m * ncols : (m + 1) * ncols],
                        ps[:, :],
                        AF.Identity,
                        bias=gb_sb[:, m : m + 1],
                        scale=1.0,
                    )

            def gru_level(B, h_tile, h16_tile, acc_tile, gi_rz_at, gi_n_at, zero_h0):
                """8 GRU steps.  h_tile [P, KT*B] fp32 state, h16_tile bf16
                shadow feeding the matmuls, acc_tile fp32 output accumulator.
                gi_rz_at(t) -> [P, 8, B] AP, gi_n_at(t) -> [P, 4, B] AP."""
                for t in range(T):
                    if t == 0 and zero_h0:
                        # h = 0 so gh == b_hh exactly; skip the matmuls.
                        rzt = wpool.tile([P, 8 * B], FP, tag="rz")
                        nc.scalar.activation(_v(rzt[:], 8), gi_rz_at(t), AF.Sigmoid)
                        bt = wpool.tile([P, KT * B], FP, tag="bt")
                        nc.vector.tensor_mul(
                            _v(bt[:], KT),
                            _v(rzt[:, : KT * B], KT),
                            _v(bhnb_sb, KT)[:, :, :B],
                        )
                        ct = wpool.tile([P, KT * B], FP, tag="ct")
                        nc.vector.tensor_add(_v(ct[:], KT), _v(bt[:], KT), gi_n_at(t))
                        nt = wpool.tile([P, KT * B], FP, tag="nt")
                        nc.scalar.activation(nt[:, :], ct[:, :], AF.Tanh)
                        # h1 = (1 - z) * n = n - z*n
                        ft = wpool.tile([P, KT * B], FP, tag="ft")
                        nc.vector.tensor_mul(ft[:, :], rzt[:, KT * B :], nt[:, :])
                        nc.vector.tensor_sub(h16_tile[:, :], nt[:, :], ft[:, :])
                        nc.vector.tensor_sub(h_tile[:, :], nt[:, :], ft[:, :])
                        nc.vector.tensor_copy(acc_tile[:, :], h_tile[:, :])
                        continue

                    # One PSUM bank per quarter so the streaming DVE/ACT
                    # reads never touch a bank the PE is still writing
                    # (same-bank PE-write / DVE-read pairs get serialized).
                    ps_rza = rzapool.tile([P, 4 * B], FP, tag="ps_rza")
                    ps_rzb = rzbpool.tile([P, 4 * B], FP, tag="ps_rzb")
                    ps_na = napool.tile([P, 2 * B], FP, tag="ps_na")
                    ps_nb = nbpool.tile([P, 2 * B], FP, tag="ps_nb")
                    arz = wpool.tile([P, 8 * B], FP, tag="arz")
                    rzt = wpool.tile([P, 8 * B], FP, tag="rz")
                    rb = wpool.tile([P, KT * B], FP, tag="rb")
                    bt = wpool.tile([P, KT * B], FP, tag="bt")
                    ct = wpool.tile([P, KT * B], FP, tag="ct")
                    nt = wpool.tile([P, KT * B], FP, tag="nt")
                    gi_rz = gi_rz_at(t)
                    gi_n = gi_n_at(t)
                    # r,z slices first; gi+gh adds and sigmoids stream behind
                    # the burst, chunk by chunk, as their banks complete.
                    for m in range(MT):
                        if m < 4:
                            dst = ps_rza[:, m * B : (m + 1) * B]
                        elif m < 8:
                            dst = ps_rzb[:, (m - 4) * B : (m - 3) * B]
                        elif m < 10:
                            dst = ps_na[:, (m - 8) * B : (m - 7) * B]
                        else:
                            dst = ps_nb[:, (m - 10) * B : (m - 9) * B]
                        for k in range(KT):
                            nc.tensor.matmul(
                                dst,
                                lhsT=wht_sb[:, (m * KT + k) * P : (m * KT + k + 1) * P],
                                rhs=h16_tile[:, k * B : (k + 1) * B],
                                start=(k == 0),
                                stop=(k == KT - 1),
                            )
                        if m == 3 or m == 7:
                            lo = 0 if m == 3 else 4
                            src = ps_rza if m == 3 else ps_rzb
                            nc.vector.tensor_add(
                                _v(arz[:, lo * B : (lo + 4) * B], 4),
                                _v(src[:], 4),
                                gi_rz[:, lo : lo + 4],
                            )
                            nc.scalar.activation(
                                rzt[:, lo * B : (lo + 4) * B],
                                arz[:, lo * B : (lo + 4) * B],
                                AF.Sigmoid,
                            )
                            if m == 7:
                                # rb = r*b_hn + gi_n (hidden under the n-gate
                                # matmuls); b_hn as a broadcast tensor keeps
                                # it to two full-width ops.
                                nc.vector.tensor_mul(
                                    _v(bt[:], KT),
                                    _v(rzt[:, : KT * B], KT),
                                    _v(bhnb_sb, KT)[:, :, :B],
                                )
                                nc.vector.tensor_add(
                                    _v(rb[:], KT), _v(bt[:], KT), gi_n
                                )
                    # b = gh_n * r + rb;  n = tanh(b)
                    bn = wpool.tile([P, KT * B], FP, tag="bn")
                    nc.vector.tensor_mul(
                        bn[:, 0 : 2 * B], ps_na[:], rzt[:, 0 : 2 * B]
                    )
                    nc.vector.tensor_mul(
                        bn[:, 2 * B : KT * B], ps_nb[:], rzt[:, 2 * B : KT * B]
                    )
                    nc.vector.tensor_add(ct[:, :], bn[:, :], rb[:, :])
                    nc.scalar.activation(nt[:, :], ct[:, :], AF.Tanh)
                    # h' = n + z*(h - n); bf16 shadow written first (it gates
                    # the next burst), fp32 state + acc follow during it on
                    # the otherwise-idle gpsimd engine.
                    et = wpool.tile([P, KT * B], FP, tag="et")
                    nc.vector.tensor_sub(et[:, :], h_tile[:, :], nt[:, :])
                    ft = wpool.tile([P, KT * B], FP, tag="ft")
                    nc.vector.tensor_mul(ft[:, :], rzt[:, KT * B :], et[:, :])
                    nc.vector.tensor_add(h16_tile[:, :], nt[:, :], ft[:, :])
                    slow_eng = nc.gpsimd if t < T - 1 else nc.vector
                    slow_eng.tensor_add(h_tile[:, :], nt[:, :], ft[:, :])
                    if t == 0:
                        slow_eng.tensor_copy(acc_tile[:, :], h_tile[:, :])
                    else:
                        slow_eng.tensor_add(
                            acc_tile[:, :], acc_tile[:, :], h_tile[:, :]
                        )

            # ---------------- Level 3: 64 nodes ----------------
            gi3 = cpool.tile([P, MT * T * NB], FP)
            compute_gi(gi3, lambda k: xt_sb[:, k * (T * NB) : (k + 1) * (T * NB)], T * NB)
            gi3v = gi3[:].rearrange("p (m t b) -> p m t b", m=MT, t=T)
            h3 = spool.tile([P, KT * NB], FP)
            h3s = spool.tile([P, KT * NB], BF)
            acc3 = spool.tile([P, KT * NB], FP)
            gru_level(
                NB, h3, h3s, acc3,
                lambda t: gi3v[:, 0:8, t],
                lambda t: gi3v[:, 8:12, t],
                zero_h0=True,
            )

            # ---------------- Level 3 -> 2 transition ----------------
            x2 = spool.tile([P, KT * NB], BF)
            nc.scalar.mul(x2[:, :], acc3[:, :], 1.0 / A)
            hr2 = spool.tile([P, KT * A], FP)
            nc.vector.tensor_reduce(
                _v(hr2[:], KT),
                h3[:].rearrange("p (k j c) -> p k j c", k=KT, j=A),
                axis=mybir.AxisListType.X,
                op=OP.add,
            )
            h2 = spool.tile([P, KT * A], FP)
            nc.scalar.mul(h2[:, :], hr2[:, :], 1.0 / A)
            h2s = spool.tile([P, KT * A], BF)
            nc.scalar.mul(h2s[:, :], hr2[:, :], 1.0 / A)

            gi2 = cpool.tile([P, MT * NB], FP)
            compute_gi(gi2, lambda k: x2[:, k * NB : (k + 1) * NB], NB)
            # gi2 within-m column order is (j, t); step-t slices are strided.
            gi2v = gi2[:].rearrange("p (m j t) -> p m j t", m=MT, j=A)
            acc2 = spool.tile([P, KT * A], FP)
            gru_level(
                A, h2, h2s, acc2,
                lambda t: gi2v[:, 0:8, :, t],
                lambda t: gi2v[:, 8:12, :, t],
                zero_h0=False,
            )

            # ---------------- Level 2 -> 1 transition ----------------
            x1 = spool.tile([P, KT * A], BF)
            nc.scalar.mul(x1[:, :], acc2[:, :], 1.0 / A)
            hr1 = spool.tile([P, KT], FP)
            nc.vector.tensor_reduce(
                _v(hr1[:], KT),
                h2[:].rearrange("p (k j c) -> p k j c", k=KT, j=1),
                axis=mybir.AxisListType.X,
                op=OP.add,
            )
            h1 = spool.tile([P, KT], FP)
            nc.scalar.mul(h1[:, :], hr1[:, :], 1.0 / A)
            h1s = spool.tile([P, KT], BF)
            nc.scalar.mul(h1s[:, :], hr1[:, :], 1.0 / A)

            gi1 = cpool.tile([P, MT * A], FP)
            compute_gi(gi1, lambda k: x1[:, k * A : (k + 1) * A], A)
            gi1v = gi1[:].rearrange("p (m t) -> p m t", m=MT)
            acc1 = spool.tile([P, KT], FP)
            gru_level(
                1, h1, h1s, acc1,
                lambda t: gi1v[:, 0:8, t : t + 1],
                lambda t: gi1v[:, 8:12, t : t + 1],
                zero_h0=False,
            )

            out_sb = spool.tile([P, KT], FP)
            nc.scalar.mul(out_sb[:, :], acc1[:, :], 1.0 / A)
            nc.sync.dma_start(out=outp[:, :], in_=out_sb[:, :])

    nc.finalize()
    return nc


def _get_nc():
    global _BUILT
    if _BUILT is None:
        _BUILT = _build_nc()
    return _BUILT


def make_inputs(leaf_ids, embed_table, W_ih, W_hh, b_ih, b_hh):
    """Host-side shard/layout prep: gather the looked-up embedding rows and
    lay every tensor out in the on-chip transposed format."""
    import ml_dtypes

    leaf_ids = np.asarray(leaf_ids).astype(np.int64)
    emb = np.asarray(embed_table, dtype=np.float32)
    W_ih = np.asarray(W_ih, dtype=np.float32)
    W_hh = np.asarray(W_hh, dtype=np.float32)
    b_ih = np.asarray(b_ih, dtype=np.float32)
    b_hh = np.asarray(b_hh, dtype=np.float32)

    x = emb[leaf_ids]  # [64, 8, 512]
    # time-major batch: row b = t*64 + node
    xtm = np.ascontiguousarray(x.transpose(1, 0, 2)).reshape(T * NB, D)
    xt_in = np.ascontiguousarray(
        xtm.T.reshape(KT, P, T * NB).transpose(1, 0, 2)
    ).reshape(P, KT * T * NB)

    def pack_w(W):  # W [1536, 512] -> lhsT tiles [(m,k) major]
        WT = np.ascontiguousarray(W.T)  # [512, 1536]
        return np.ascontiguousarray(
            WT.reshape(KT, P, MT, P).transpose(1, 2, 0, 3)
        ).reshape(P, MT * KT * P)

    blob16 = np.concatenate([xt_in, pack_w(W_ih), pack_w(W_hh)], axis=1).astype(
        ml_dtypes.bfloat16
    )

    gbias = np.concatenate([(b_ih + b_hh)[: 2 * D], b_ih[2 * D :]])
    gb_in = np.ascontiguousarray(gbias.reshape(MT, P).T)
    bhn_in = np.ascontiguousarray(b_hh[2 * D :].reshape(KT, P).T)
    bhnb_in = np.ascontiguousarray(np.repeat(bhn_in, NB, axis=1))
    blob32 = np.concatenate([gb_in, bhn_in, bhnb_in], axis=1)

    assert blob16.shape == (P, B16_COLS) and blob32.shape == (P, B32_COLS)
    return {
        "blob16": np.ascontiguousarray(blob16),
        "blob32": np.ascontiguousarray(blob32),
    }


def unpack_output(out_np):
    # out [P, KT]: element (p, k) = root dim k*128+p
    return np.ascontiguousarray(out_np.T).reshape(1, 1, D).astype(np.float32)


def kernel(leaf_ids=None, layer=None, embed_table=None, W_ih=None, W_hh=None,
           b_ih=None, b_hh=None, **_unused):
    in_map = make_inputs(leaf_ids, embed_table, W_ih, W_hh, b_ih, b_hh)
    nc = _get_nc()
    res = run_bass_kernel_spmd(nc, [in_map] * N_CORES, list(range(N_CORES)))
    return unpack_output(res.results[0]["out"])
